# revision 1
# baseline (speedup 1.0000x reference)
"""Attention-pooling Trainium2 kernel.

Problem: out = mean_s(softmax((x@Wq+bq)(x@Wk+bk)^T / sqrt(E)) @ (x@Wv+bv))
with x [4, 4096, 256], output [4, 1, 256].

Math restructuring (all exact up to fp reassociation):
  * mean_s(dist @ V) = (colsum(dist)/S) @ V  -- the second S x S matmul
    collapses to a length-S vector "w" and one matvec.
  * K bias drops entirely (adds a per-row constant to scores; softmax is
    row-shift invariant). V bias folds to a host-side "+bv" (rows of dist
    sum to 1). Only the Q bias is applied on-device.
  * No per-row max subtraction before exp: scores here are ~N(0,1) and
    fp32 exp is exact enough; this removes a full-row dependency.

Sharding: 8 cores = 4 batches x 2 query-row halves. Each core receives
x[b].T *rolled* so its own 2048 query rows are always columns 0:2047 --
the key/value permutation is harmless because w@V is permutation
invariant. Cores compute partial w @ V; the host sums the two halves per
batch, divides by S, and adds bv.

Per-core pipeline (Tile framework, bf16 matmul inputs, fp32 PSUM):
  PE:  Q^T/K^T projections, V in [t,e] layout, scores per 128-row q-tile,
       colsum matmuls (lhsT=recip[128,1]) accumulating w into PSUM column
       strips (4-way col-group concurrency), w transposes, final matvec
       w^T x V in 4 concurrent strips + mask-matmul combine.
  ACT: exp(scale=1/16) into E (bf16), Q-bias copies, some rowsums via the
       activation accumulator.
  DVE: projection PSUM->SBUF copies, remaining rowsum reduces,
       reciprocals, small glue.
Measured ~126 us on hardware per core (span), PE-bound at ~97 us busy.
"""

import numpy as np

import concourse.bass as bass  # noqa: F401
import concourse.mybir as mybir
import concourse.tile as tile
from concourse import bacc

B, S, E = 4, 4096, 256
HALF = S // 2          # query rows per core
NORM = 16.0            # sqrt(E)
P = 128
N_CORES = 8
QTILES = HALF // P     # 16
F32 = mybir.dt.float32
BF16 = mybir.dt.bfloat16

EXPW = 1024            # exp() chunk width (PSUM tile width)
NEXP = S // EXPW       # 4 chunks per q-tile
XTW = 2048             # xt load chunk width (big DMAs; few per queue)
NXQ = S // XTW
PROJW = 1024           # projection unit width


def _emit(ctx, tc):
    nc = tc.nc

    xt_d = nc.dram_tensor("xt", [E, S], BF16, kind="ExternalInput")
    # all six weight chunks stacked: [(wq0,wq1,wk0,wk1,wv0,wv1), 128, 256]
    wall_d = nc.dram_tensor("wall", [6, P, E], BF16, kind="ExternalInput")
    bqc_d = nc.dram_tensor("bqc", [E, 1], F32, kind="ExternalInput")
    out_d = nc.dram_tensor("out", [4, E], F32, kind="ExternalOutput")

    const = ctx.enter_context(tc.tile_pool(name="const", bufs=1))
    epool = ctx.enter_context(tc.tile_pool(name="epool", bufs=4))
    rsp = ctx.enter_context(tc.tile_pool(name="rsp", bufs=3))
    pp = ctx.enter_context(tc.tile_pool(name="pp", bufs=3, space="PSUM"))
    wp = ctx.enter_context(tc.tile_pool(name="wp", bufs=1, space="PSUM"))

    # ---- small loads first so projections can start immediately.
    # One DMA for all six weight chunks; the first xt columns split into
    # small 1024-wide tiles (one per queue) so the first Q^T matmul and the
    # first scores tile are unblocked as early as possible.
    wq_all = const.tile([P, 2, E], BF16, name="wq_all")
    wkv_all = const.tile([P, 4, E], BF16, name="wkv_all")
    chunk_bounds = [(0, 512), (512, 512), (1024, 1024)] + [
        (s, XTW) for s in range(2048, S, XTW)
    ]
    xt_sb = [[None] * len(chunk_bounds) for _ in range(2)]

    def xt_load(ci, ei, eng):
        t0, width = chunk_bounds[ci]
        t = const.tile([P, width], BF16, name=f"xt{ei}_{ci}", tag=f"xt{ei}_{ci}")
        eng.dma_start(out=t, in_=xt_d[ei * P : (ei + 1) * P, t0 : t0 + width])
        xt_sb[ei][ci] = t

    nc.sync.dma_start(
        out=wq_all, in_=wall_d[0:2, :, :].rearrange("s p e -> p s e")
    )
    xt_load(0, 0, nc.scalar)
    xt_load(0, 1, nc.sync)
    xt_load(1, 0, nc.scalar)
    xt_load(1, 1, nc.sync)
    nc.scalar.dma_start(
        out=wkv_all, in_=wall_d[2:6, :, :].rearrange("s p e -> p s e")
    )
    bqc_sb = const.tile([P, 2], F32, name="bqc_sb")
    nc.sync.dma_start(
        out=bqc_sb, in_=bqc_d[:, :].rearrange("(a p) one -> p (a one)", a=2)
    )
    for ci in range(2, len(chunk_bounds)):
        xt_load(ci, 0, nc.scalar if ci % 2 else nc.sync)
        xt_load(ci, 1, nc.sync if ci % 2 else nc.scalar)

    w_sb = {
        "wq": [wq_all[:, ei, :] for ei in range(2)],
        "wk": [wkv_all[:, ei, :] for ei in range(2)],
        "wv": [wkv_all[:, 2 + ei, :] for ei in range(2)],
    }
    identity = const.tile([P, P], F32, name="identity")
    from concourse.masks import make_identity

    make_identity(nc, identity)

    # ---- projections
    qt_sb = [const.tile([P, HALF], BF16, name=f"qt{eo}") for eo in range(2)]
    kt_sb = [const.tile([P, S], BF16, name=f"kt{eo}") for eo in range(2)]
    # V in [t, e] layout: v_sb[:, n, :] = V[128n:128n+128, :]
    v_sb = const.tile([P, S // P, E], BF16, name="v_sb")

    def xt_slice(ei, t0, width):
        for ci, (c0, cw) in enumerate(chunk_bounds):
            if c0 <= t0 and t0 + width <= c0 + cw:
                return xt_sb[ei][ci][:, t0 - c0 : t0 - c0 + width]
        raise AssertionError(f"xt slice [{t0}, {t0+width}) crosses chunk bounds")

    def proj_unit(dst, wname, eo, u, act_bias=None):
        """dst[:, u*PROJW:(u+1)*PROJW] = (W[:,eo-chunk].T @ xt chunk) (+bias)."""
        ps = pp.tile([P, EXPW], F32, tag="ps", name=f"ps_{wname}{eo}_{u}")
        for ei in range(2):
            lhsT = w_sb[wname][ei][:, eo * P : (eo + 1) * P]
            for h in range(PROJW // 512):
                nc.tensor.matmul(
                    ps[:, h * 512 : (h + 1) * 512],
                    lhsT,
                    xt_slice(ei, u * PROJW + h * 512, 512),
                    start=(ei == 0),
                    stop=(ei == 1),
                )
        dv = dst[:, u * PROJW : (u + 1) * PROJW]
        if act_bias is not None:
            nc.scalar.activation(
                out=dv,
                in_=ps,
                func=mybir.ActivationFunctionType.Identity,
                bias=act_bias,
                scale=1.0,
            )
        else:
            nc.vector.tensor_copy(out=dv, in_=ps)

    for eo in range(2):
        for u in range(HALF // PROJW):
            proj_unit(qt_sb[eo], "wq", eo, u, act_bias=bqc_sb[:, eo : eo + 1])
    for eo in range(2):
        for u in range(S // PROJW):
            proj_unit(kt_sb[eo], "wk", eo, u)

    def v_unit(u):
        """V rows for t-tiles 2u, 2u+1 -> v_sb[:, 2u:2u+2, :]."""
        ps = pp.tile([P, EXPW], F32, tag="ps", name=f"ps_v{u}")
        for a in range(2):
            t0 = (2 * u + a) * P
            for ei in range(2):
                nc.tensor.matmul(
                    ps[:, a * E : (a + 1) * E],
                    xt_slice(ei, t0, P),
                    w_sb["wv"][ei],
                    start=(ei == 0),
                    stop=(ei == 1),
                )
        nc.vector.tensor_copy(out=v_sb[:, 2 * u : 2 * u + 2, :], in_=ps[:, : 2 * E])

    # ---- w accumulators: chunk m (512 wide) -> tile m//4, strip 32*(m%4).
    # memset the full tiles so non-strip partitions are defined: the tail
    # can then drain each tile with one full-width DVE copy.
    w_ps = [wp.tile([P, 512], F32, tag=f"w{i}", name=f"w_ps{i}") for i in range(2)]
    for i in range(2):
        nc.vector.memset(w_ps[i], 0.0)

    def w_slot(m):
        return w_ps[m // 4], 32 * (m % 4)

    def emit_scores(qi):
        # rowsum load-balancing: some q-tiles get their row sums from ACT's
        # per-chunk accumulator (cheap reads), the rest from one DVE reduce
        act_rowsum = qi % 2 == 0 or qi >= 14
        Es = epool.tile([P, S], BF16, tag="E", name=f"E{qi}")
        rs = None
        if act_rowsum:
            rs = rsp.tile([P, NEXP], F32, tag="rs", name=f"rs{qi}")
        for tc4 in range(NEXP):
            ps = pp.tile([P, EXPW], F32, tag="ps", name=f"ps_s{qi}_{tc4}")
            for eo in range(2):
                lhsT = qt_sb[eo][:, qi * P : (qi + 1) * P]
                for h in range(EXPW // 512):
                    t0 = tc4 * EXPW + h * 512
                    nc.tensor.matmul(
                        ps[:, h * 512 : (h + 1) * 512],
                        lhsT,
                        kt_sb[eo][:, t0 : t0 + 512],
                        start=(eo == 0),
                        stop=(eo == 1),
                    )
            nc.scalar.activation(
                out=Es[:, tc4 * EXPW : (tc4 + 1) * EXPW],
                in_=ps,
                func=mybir.ActivationFunctionType.Exp,
                scale=1.0 / NORM,
                accum_out=rs[:, tc4 : tc4 + 1] if act_rowsum else None,
            )
        rsum = rsp.tile([P, 1], F32, tag="rsum", name=f"rsum{qi}")
        if act_rowsum:
            nc.vector.reduce_sum(out=rsum, in_=rs, axis=mybir.AxisListType.X)
        else:
            nc.vector.reduce_sum(out=rsum, in_=Es, axis=mybir.AxisListType.X)
        recf = rsp.tile([P, 1], F32, tag="recf", name=f"recf{qi}")
        nc.vector.reciprocal(out=recf, in_=rsum)
        recb = rsp.tile([P, 1], BF16, tag="recb", name=f"recb{qi}")
        nc.vector.tensor_copy(out=recb, in_=recf)
        return Es, recb

    def emit_colsum(qi, Es, recb):
        for m in range(8):
            wt, strip = w_slot(m)
            nc.tensor.matmul(
                wt[strip : strip + 1, :],
                recb,
                Es[:, m * 512 : (m + 1) * 512],
                start=(qi == 0),
                stop=(qi == QTILES - 1),
                tile_position=(0, strip),
            )

    # software-pipeline colsum by 2 q-tiles: its recip dependency chain
    # (DVE reduce -> reciprocal -> cast) takes ~5us, one tile isn't enough
    pending = {}
    for qi in range(QTILES):
        pending[qi] = emit_scores(qi)
        v_unit(qi)
        if qi - 3 in pending:
            emit_colsum(qi - 3, *pending.pop(qi - 3))
    for qi in sorted(pending):
        emit_colsum(qi, *pending.pop(qi))

    # ---- tail: w strips -> SBUF, PE-transpose to w^T, matvec against V.
    # w_sb_f[strip 32k, 512a + u] = w[t = 2048a + 512k + u]
    w_sb_f = const.tile([P, 1024], F32, name="w_sb_f")
    nc.vector.tensor_copy(out=w_sb_f[:, 0:512], in_=w_ps[0])
    nc.scalar.copy(out=w_sb_f[:, 512:1024], in_=w_ps[1])
    # transpose each 128-col block; valid rows of the result sit at
    # partitions {0,32,64,96} = k -> wt_sb col c*4+k pairs with
    # v tile index 16*(c//4) + 4*k + (c%4)
    wt_sb = const.tile([P, 32], BF16, name="wt_sb")
    for c in range(8):
        tp = pp.tile([P, EXPW], F32, tag="ps", name=f"tp{c}")
        nc.tensor.transpose(
            out=tp[:, 0:P], in_=w_sb_f[:, c * P : (c + 1) * P], identity=identity
        )
        src = bass.AP(
            tensor=tp.tensor,
            offset=tp.offset,
            ap=[tp.ap[0], [32, 4]],
        )
        nc.vector.tensor_copy(out=wt_sb[:, c * 4 : (c + 1) * 4], in_=src)
    # final matvec: 4 concurrent column strips accumulate partial sums at
    # partitions {0,32,64,96}; combine with a mask-weighted matmul
    fin = wp.tile([P, 512], F32, tag="w0", name="fin")
    nc.vector.memset(fin[:, 0:E], 0.0)
    for col in range(32):
        c, k = divmod(col, 4)
        vidx = 16 * (c // 4) + 4 * k + (c % 4)
        strip = 32 * (col % 4)
        nc.tensor.matmul(
            fin[strip : strip + 1, 0:E],
            wt_sb[:, col : col + 1],
            v_sb[:, vidx, :],
            start=(col < 4),
            stop=(col >= 28),
            tile_position=(0, strip),
        )
    # the 4 strip partials go back to the host, which sums them -- this
    # drops a matmul + copy + semaphore hop from the serial tail
    strips_sb = const.tile([P, E], F32, name="strips_sb")
    nc.vector.tensor_copy(out=strips_sb, in_=fin[:, 0:E])
    src_ap = bass.AP(
        tensor=strips_sb.tensor,
        offset=strips_sb.offset,
        ap=[[strips_sb.ap[0][0] * 32, 4], [1, E]],
    )
    nc.sync.dma_start(out=out_d[:, :], in_=src_ap)


_NC_CACHE = None


def _build_nc():
    global _NC_CACHE
    if _NC_CACHE is None:
        from contextlib import ExitStack

        nc = bacc.Bacc("TRN2", target_bir_lowering=False, debug=False)
        with tile.TileContext(nc) as tc, ExitStack() as ctx:
            _emit(ctx, tc)
        nc.compile()
        _NC_CACHE = nc
    return _NC_CACHE


def _in_maps(inputs):
    import ml_dtypes

    bf16 = ml_dtypes.bfloat16
    x = np.asarray(inputs["x"], dtype=np.float32)
    wall = np.empty((6, P, E), dtype=bf16)
    for i, nm in enumerate(("Wq", "Wk", "Wv")):
        w = np.asarray(inputs[nm], dtype=np.float32)
        wall[2 * i] = w[:P, :].astype(bf16)
        wall[2 * i + 1] = w[P:, :].astype(bf16)
    bqc = np.ascontiguousarray(
        np.asarray(inputs["bq"], dtype=np.float32).reshape(E, 1)
    )
    maps = []
    for c in range(N_CORES):
        b, h = divmod(c, 2)
        xt = np.ascontiguousarray(np.roll(x[b], -h * HALF, axis=0).T).astype(bf16)
        maps.append({"xt": xt, "wall": wall, "bqc": bqc})
    return maps


def _combine(results, inputs):
    bv = np.asarray(inputs["bv"], dtype=np.float32).reshape(E)
    parts = [r["out"].sum(axis=0) for r in results]
    out = np.stack(
        [(parts[2 * b] + parts[2 * b + 1]) / S + bv for b in range(B)]
    )[:, None, :]
    return out.astype(np.float32)


def kernel(**inputs):
    from concourse.bass_utils import run_bass_kernel_spmd

    nc = _build_nc()
    res = run_bass_kernel_spmd(nc, _in_maps(inputs), core_ids=list(range(N_CORES)))
    return _combine(res.results, inputs)



# revision 4
# speedup vs baseline: 1.1898x; 1.1898x over previous
"""Attention-pooling Trainium2 kernel (fp8 DoubleRow + split-engine exp).

Problem: out = mean_s(softmax((x@Wq+bq)(x@Wk+bk)^T / sqrt(E)) @ (x@Wv+bv))
with x [4, 4096, 256], output [4, 1, 256].

Math restructuring (exact up to fp reassociation):
  * mean_s(dist @ V) = (colsum(dist)/S) @ V  -- the second S x S matmul
    collapses to a length-S vector "w" and one matvec.
  * K bias drops (row-constant in scores); V bias folds to host "+bv".
  * Q/K projections fold into ONE projection: scores = x M x^T + u^T x^T
    with M = Wq Wk^T, u = Wk bq (host-computed E x E / E-sized weight prep).
    So the device never computes K.
  * Wv moves to the END: pooled = (w @ x) @ Wv -- the V projection
    (S x E x E) becomes an E x E matmul on a [1, E] vector.
  * Scores run in fp8(e4m3) with DoubleRow perf mode: the E=256
    contraction happens in ONE PE pass at 2x bf16 rate. M is pre-scaled
    by 16 host-side so fp8 operands sit in their sweet spot; the exp
    applies scale 1/256 and a constant -2 shift to keep exp outputs in
    range. Numerics validated in simulation: rel_err ~0.009 vs 2e-2 gate.
  * exp is split across engines: ACT computes true exp (with accum_out
    row-sums); DVE computes a Schraudolph-style exp -- one tensor_scalar
    (score*A + B) -> int16, whose bit pattern IS the bf16 exp
    approximation (+-3.5% sawtooth, washes out in the pooled mean).

Sharding: 8 cores = 4 batches x 2 query-row halves; x arrives rolled so
each core's 2048 query rows are columns 0:2047 (permutation-invariant
for the pooled result). Host sums the two halves per batch, /S, +bv.
"""

import numpy as np

import concourse.bass as bass  # noqa: F401
import concourse.mybir as mybir
import concourse.tile as tile
from concourse import bacc

B, S, E = 4, 4096, 256
HALF = S // 2          # query rows per core
P = 128
N_CORES = 8
QTILES = HALF // P     # 16
F32 = mybir.dt.float32
BF16 = mybir.dt.bfloat16
FP8 = mybir.dt.float8e4
I16 = mybir.dt.int16
DR = mybir.MatmulPerfMode.DoubleRow

CSHIFT = 2.0                       # exp(score - CSHIFT): keeps e4m3/bf16 in range
A_SCH = 128.0 / np.log(2.0)        # bf16 Schraudolph slope (per unit exp arg)
A2 = A_SCH / 256.0                 # folded score scale 1/256
B2 = (127 * 128 - 5.5) - CSHIFT * A_SCH
COLSUM_LAG = 3
# which 1024-wide chunks of each q-tile's exp go to DVE (rest go to ACT)
DVE_CHUNKS = [(3,) if qi % 2 == 0 else (2, 3) for qi in range(QTILES)]


def _emit(ctx, tc):
    nc = tc.nc

    x8_d = nc.dram_tensor("x8", [P, 2, S], FP8, kind="ExternalInput")
    xte_d = nc.dram_tensor("xte", [P, S // P, E], BF16, kind="ExternalInput")
    m8_d = nc.dram_tensor("m8", [P, 2, E], FP8, kind="ExternalInput")
    wvb_d = nc.dram_tensor("wvb", [P, 2, E], BF16, kind="ExternalInput")
    u16_d = nc.dram_tensor("u16c", [P, 2], F32, kind="ExternalInput")
    out_d = nc.dram_tensor("out", [P, 2], F32, kind="ExternalOutput")

    const = ctx.enter_context(tc.tile_pool(name="const", bufs=1))
    epool = ctx.enter_context(tc.tile_pool(name="epool", bufs=COLSUM_LAG + 1))
    rsp = ctx.enter_context(tc.tile_pool(name="rsp", bufs=COLSUM_LAG + 2))
    pp = ctx.enter_context(tc.tile_pool(name="pp", bufs=3, space="PSUM"))
    wp = ctx.enter_context(tc.tile_pool(name="wp", bufs=1, space="PSUM"))

    # ---- small loads first so the q' projection can start immediately.
    m8 = const.tile([P, 2, E], FP8, name="m8")
    u16 = const.tile([P, 2], F32, name="u16")
    nc.sync.dma_start(out=m8, in_=m8_d[:, :, :])
    nc.sync.dma_start(out=u16, in_=u16_d[:, :])

    # x^T in fp8, [e-part, e-chunk-plane, t] -- DoubleRow rhs layout.
    bounds = [(0, 512), (512, 512), (1024, 1024), (2048, 1024), (3072, 1024)]
    x8c = [None] * len(bounds)
    for i, (c0, w) in enumerate(bounds):
        t = const.tile([P, 2, w], FP8, name=f"x8_{i}", tag=f"x8_{i}")
        eng = nc.scalar if i % 2 else nc.sync
        eng.dma_start(out=t, in_=x8_d[:, :, c0 : c0 + w])
        x8c[i] = t

    def x8s(t0, width):
        for i, (c0, cw) in enumerate(bounds):
            if c0 <= t0 and t0 + width <= c0 + cw:
                return x8c[i][:, :, t0 - c0 : t0 - c0 + width]
        raise AssertionError(f"x8 slice [{t0}, {t0+width}) crosses chunk bounds")

    # x rows in bf16, [t-part, t-tile, e] -- final matvec rhs (tail only).
    xte = const.tile([P, S // P, E], BF16, name="xte")
    nc.scalar.dma_start(out=xte, in_=xte_d[:, :, :])
    wvb = const.tile([P, 2, E], BF16, name="wvb")
    nc.sync.dma_start(out=wvb, in_=wvb_d[:, :, :])

    identity = const.tile([P, P], F32, name="identity")
    from concourse.masks import make_identity

    make_identity(nc, identity)
    negc = const.tile([P, 1], F32, name="negc")
    nc.vector.memset(negc, -CSHIFT)

    # ---- q' projection: q'16^T = M16^T @ x^T + u16 (DoubleRow, fp8 out)
    q8 = const.tile([P, 2, HALF], FP8, name="q8")
    for eo in range(2):
        for half in range(2):
            ps = pp.tile([P, 1024], F32, tag="ps", name=f"ps_q{eo}_{half}")
            for h in range(2):
                c0 = half * 1024 + h * 512
                nc.tensor.matmul(
                    ps[:, h * 512 : (h + 1) * 512],
                    m8[:, :, eo * P : (eo + 1) * P],
                    x8s(c0, 512),
                    start=True,
                    stop=True,
                    perf_mode=DR,
                )
            nc.vector.tensor_scalar(
                out=q8[:, eo, half * 1024 : (half + 1) * 1024],
                in0=ps,
                scalar1=u16[:, eo : eo + 1],
                scalar2=None,
                op0=mybir.AluOpType.add,
            )

    # ---- w accumulators: key chunk m (512 wide) -> tile m//4, strip 32*(m%4)
    w_ps = [wp.tile([P, 512], F32, tag=f"w{i}", name=f"w_ps{i}") for i in range(2)]
    for i in range(2):
        nc.vector.memset(w_ps[i], 0.0)

    def emit_scores(qi):
        Es = epool.tile([P, S], BF16, tag="E", name=f"E{qi}")
        rs = rsp.tile([P, 4], F32, tag="rs", name=f"rs{qi}")
        ei16 = Es.bitcast(I16)
        for c in range(4):
            ps = pp.tile([P, 1024], F32, tag="ps", name=f"ps_s{qi}_{c}")
            for h in range(2):
                t0 = c * 1024 + h * 512
                nc.tensor.matmul(
                    ps[:, h * 512 : (h + 1) * 512],
                    q8[:, :, qi * P : (qi + 1) * P],
                    x8s(t0, 512),
                    start=True,
                    stop=True,
                    perf_mode=DR,
                )
            sl = slice(c * 1024, (c + 1) * 1024)
            if c in DVE_CHUNKS[qi]:
                nc.vector.tensor_scalar(
                    out=ei16[:, sl],
                    in0=ps,
                    scalar1=float(A2),
                    scalar2=float(B2),
                    op0=mybir.AluOpType.mult,
                    op1=mybir.AluOpType.add,
                )
                nc.vector.reduce_sum(
                    out=rs[:, c : c + 1], in_=Es[:, sl], axis=mybir.AxisListType.X
                )
            else:
                nc.scalar.activation(
                    out=Es[:, sl],
                    in_=ps,
                    func=mybir.ActivationFunctionType.Exp,
                    scale=1.0 / 256.0,
                    bias=negc,
                    accum_out=rs[:, c : c + 1],
                )
        rsum = rsp.tile([P, 1], F32, tag="rsum", name=f"rsum{qi}")
        nc.vector.reduce_sum(out=rsum, in_=rs, axis=mybir.AxisListType.X)
        recf = rsp.tile([P, 1], F32, tag="recf", name=f"recf{qi}")
        nc.vector.reciprocal(out=recf, in_=rsum)
        recb = rsp.tile([P, 1], BF16, tag="recb", name=f"recb{qi}")
        nc.vector.tensor_copy(out=recb, in_=recf)
        return Es, recb

    def emit_colsum(qi, Es, recb):
        for m in range(8):
            wt, strip = w_ps[m // 4], 32 * (m % 4)
            nc.tensor.matmul(
                wt[strip : strip + 1, :],
                recb,
                Es[:, m * 512 : (m + 1) * 512],
                start=(qi == 0),
                stop=(qi == QTILES - 1),
                tile_position=(0, strip),
            )

    pending = {}
    for qi in range(QTILES):
        pending[qi] = emit_scores(qi)
        if qi - COLSUM_LAG in pending:
            emit_colsum(qi - COLSUM_LAG, *pending.pop(qi - COLSUM_LAG))
    for qi in sorted(pending):
        emit_colsum(qi, *pending.pop(qi))

    # ---- tail: w strips -> SBUF, PE-transpose to w^T, matvec against x,
    # then apply Wv to the pooled vector on-device.
    # w_sb_f[strip 32k, 512a + u] = w[t = 2048a + 512k + u]
    w_sb_f = const.tile([P, 1024], F32, name="w_sb_f")
    nc.vector.tensor_copy(out=w_sb_f[:, 0:512], in_=w_ps[0])
    nc.scalar.copy(out=w_sb_f[:, 512:1024], in_=w_ps[1])
    wt_sb = const.tile([P, 32], BF16, name="wt_sb")
    for c in range(8):
        tp = pp.tile([P, 1024], F32, tag="ps", name=f"tp{c}")
        nc.tensor.transpose(
            out=tp[:, 0:P], in_=w_sb_f[:, c * P : (c + 1) * P], identity=identity
        )
        src = bass.AP(tensor=tp.tensor, offset=tp.offset, ap=[tp.ap[0], [32, 4]])
        nc.vector.tensor_copy(out=wt_sb[:, c * 4 : (c + 1) * 4], in_=src)
    # pooled-x partials: 4 concurrent column strips at partitions {0,32,64,96}
    fin = wp.tile([P, 512], F32, tag="w0", name="fin")
    nc.vector.memset(fin[:, 0:E], 0.0)
    for col in range(32):
        c, k = divmod(col, 4)
        vidx = 16 * (c // 4) + 4 * k + (c % 4)
        strip = 32 * (col % 4)
        nc.tensor.matmul(
            fin[strip : strip + 1, 0:E],
            wt_sb[:, col : col + 1],
            xte[:, vidx, :],
            start=(col < 4),
            stop=(col >= 28),
            tile_position=(0, strip),
        )
    strips_sb = const.tile([P, E], F32, name="strips_sb")
    nc.vector.tensor_copy(out=strips_sb, in_=fin[:, 0:E])
    # transpose the 4 strip partials to [e-part, strip] and apply Wv
    ss4 = const.tile([P, 2, 4], BF16, name="ss4")
    for ei in range(2):
        tpe = pp.tile([P, 1024], F32, tag="ps", name=f"tpe{ei}")
        nc.tensor.transpose(
            out=tpe[:, 0:P], in_=strips_sb[:, ei * P : (ei + 1) * P], identity=identity
        )
        src = bass.AP(tensor=tpe.tensor, offset=tpe.offset, ap=[tpe.ap[0], [32, 4]])
        nc.vector.tensor_copy(out=ss4[:, ei, :], in_=src)
    pf = wp.tile([P, 512], F32, tag="w1", name="pf")
    for eo in range(2):
        for ei in range(2):
            nc.tensor.matmul(
                pf[:, eo * 4 : (eo + 1) * 4],
                wvb[:, ei, eo * P : (eo + 1) * P],
                ss4[:, ei, :],
                start=(ei == 0),
                stop=(ei == 1),
            )
    po = const.tile([P, 2], F32, name="po")
    for eo in range(2):
        nc.vector.reduce_sum(
            out=po[:, eo : eo + 1],
            in_=pf[:, eo * 4 : (eo + 1) * 4],
            axis=mybir.AxisListType.X,
        )
    nc.sync.dma_start(out=out_d[:, :], in_=po)


_NC_CACHE = None


def _build_nc():
    global _NC_CACHE
    if _NC_CACHE is None:
        from contextlib import ExitStack

        nc = bacc.Bacc("TRN2", target_bir_lowering=False, debug=False)
        with tile.TileContext(nc) as tc, ExitStack() as ctx:
            _emit(ctx, tc)
        nc.compile()
        _NC_CACHE = nc
    return _NC_CACHE


def _in_maps(inputs):
    import ml_dtypes

    bf16 = ml_dtypes.bfloat16
    f8 = ml_dtypes.float8_e4m3

    def to8(a):
        return np.clip(a, -240.0, 240.0).astype(f8)

    x = np.asarray(inputs["x"], dtype=np.float32)
    Wq = np.asarray(inputs["Wq"], dtype=np.float64)
    Wk = np.asarray(inputs["Wk"], dtype=np.float64)
    Wv = np.asarray(inputs["Wv"], dtype=np.float32)
    bq = np.asarray(inputs["bq"], dtype=np.float64)

    M16 = (16.0 * (Wq @ Wk.T)).astype(np.float32)
    u16 = (16.0 * (Wk @ bq)).astype(np.float32)
    m8 = np.ascontiguousarray(to8(M16).reshape(2, P, E).transpose(1, 0, 2))
    u16c = np.ascontiguousarray(u16.reshape(2, P).T)
    wvb = np.ascontiguousarray(Wv.astype(bf16).reshape(2, P, E).transpose(1, 0, 2))

    maps = []
    for c in range(N_CORES):
        b, h = divmod(c, 2)
        xr = np.roll(x[b], -h * HALF, axis=0)
        x8 = np.ascontiguousarray(
            to8(xr.T).reshape(2, P, S).transpose(1, 0, 2)
        )
        xte = np.ascontiguousarray(
            xr.astype(bf16).reshape(S // P, P, E).transpose(1, 0, 2)
        )
        maps.append({"x8": x8, "xte": xte, "m8": m8, "wvb": wvb, "u16c": u16c})
    return maps


def _combine(results, inputs):
    bv = np.asarray(inputs["bv"], dtype=np.float32).reshape(E)
    pooled = [np.asarray(r["out"], np.float32).T.reshape(E) for r in results]
    out = np.stack(
        [(pooled[2 * b] + pooled[2 * b + 1]) / S + bv for b in range(B)]
    )[:, None, :]
    return out.astype(np.float32)


def kernel(**inputs):
    from concourse.bass_utils import run_bass_kernel_spmd

    nc = _build_nc()
    res = run_bass_kernel_spmd(nc, _in_maps(inputs), core_ids=list(range(N_CORES)))
    return _combine(res.results, inputs)
